# revision 21
# baseline (speedup 1.0000x reference)
"""Multi-head attention (B=2, S=2048, D=1024, H=16, causal) on 8 TRN2 cores.

Sharding: batch (2) x head-groups (4 heads per core). Each core:
  - projects its 4 heads' Q/K/V (fp16 matmuls, full PE rate)
  - causal flash attention in transposed layout:
      S^T[k,q] = Kt.T @ Qt  (K=64 contraction; the two heads of a pair are
            emitted as row-tiled matmuls at base partitions 0/64 so the PE
            runs them concurrently in one 512-cycle slot)
      P^T = exp(S^T/8) via ACT straight from PSUM; diagonal blocks masked in
            place with a 0/1 triangle multiply on DVE
      ctx^T+sumexp = [V | ones].T @ P^T accumulated over k-blocks in PSUM;
            the 64 ones-columns replicate sumexp across partitions (free in
            PE time: matmul cost is N cycles regardless of M) so the
            normalize is reciprocal (ACT ln+exp) + plain multiplies (DVE)
  - partial out-projection out_c = ctx_norm^T.T @ Wo[slice]
Host: out[b] = sum over the batch's 4 cores + bo + bv @ Wo.

Schedule details (all measured off perfetto traces):
  - ~10 dummy matmuls on a memset scratch tile run during the DMA spin-up so
    the PE_HAM clock gate un-throttles (1.2 -> 2.4 GHz) before real work.
  - weight/x DMAs are chunked and ordered so the first projection's
    dependencies land ~1us after the HWDGE ring starts; wv/xk/xv/tri/wo ride
    the ACT ring in parallel; outputs ride the idle gpsimd SWDGE ring.
  - the ones-columns are written by a single strided DVE memset (the DMA
    version costs ~8k descriptors on the sync ring).
  - per head-pair the scores matmul for k-block kb+1 is emitted BEFORE the
    PV matmul of kb: PV waits on the ACT exp, and the PE queue is in-order,
    so this keeps a score slot runnable while exp catches up.
  - tile-3's first head-pair attention is pulled one phase earlier to spread
    the terminal ACT(exp)-heavy stretch across more PE work.
"""
import sys

sys.path.insert(0, "/opt/trn_rl_repo")

import numpy as np
import concourse.bass as bass
import concourse.tile as tile
import concourse.mybir as mybir
from concourse.bass_utils import run_bass_kernel_spmd
B, S, D, NH, HD = 2, 2048, 1024, 16, 64
NCORE = 8
HPC = NH // (NCORE // B)      # heads per core = 4
DOUT = HPC * HD               # 256 per-core projection width
NT = 4                        # seq tiles of 512
TW = S // NT                  # 512
NKB = S // 128                # 16 k-blocks
KPC = D // 128                # 8 contraction chunks for projections
NWARM = 8

f32 = mybir.dt.float32
# fp16 (10-bit mantissa) streams 1 row/cycle on the PE and gets Fast Weight
# Load; fp32r needs 2 half-rate passes. End-to-end error stays ~5e-4.
fmm = mybir.dt.float16
EXP = mybir.ActivationFunctionType.Exp
LN = mybir.ActivationFunctionType.Ln


def _split_sync_waits(nc):
    """walrus rejects >1 sync wait on most instructions; hoist extras onto
    preceding NoOps on the same engine (sems are monotone, so waiting
    earlier is always safe)."""
    for func in nc.m.functions:
        for blk in func.blocks:
            insts = list(blk.instructions)
            out = []
            changed = False
            for inst in insts:
                si = inst.sync_info
                waits = list(si.on_wait) if (si is not None and si.on_wait) else []
                if len(waits) > 1:
                    hoist, keep = waits[:-1], waits[-1:]
                    for i, w in enumerate(hoist):
                        nop = mybir.InstNoOp(
                            name=f"{inst.name}-ws{i}",
                            engine=inst.engine,
                            sync_info=mybir.SyncInfo(on_wait=[w], on_update=[]),
                        )
                        nop.bass_nofuse = True
                        out.append(nop)
                    inst.sync_info = mybir.SyncInfo(
                        on_wait=keep, on_update=list(si.on_update)
                    )
                    changed = True
                out.append(inst)
            if changed:
                blk.instructions = out


def _act_recip(nc, out, in_, tmp):
    # 1/x = exp(-ln(x)). Ln and Exp share one ACT table set
    # (natural_log_exp_and_others), so this costs two streaming passes and
    # zero table reloads.
    nc.scalar.activation(tmp, in_, LN)
    nc.scalar.activation(out, tmp, EXP, scale=-1.0)


def _weighted_merge(la, lb):
    out = []
    ia = ib = 0
    na, nb = len(la), len(lb)
    while ia < na or ib < nb:
        if ib >= nb or (ia < na and ia * nb <= ib * na):
            out.append(la[ia]); ia += 1
        else:
            out.append(lb[ib]); ib += 1
    return out


def _tail_merge(lb, fill):
    """Merge filler into the b-stream with density increasing toward the
    end: ACT (exp) lag accumulates over a phase, so PE-ready filler is worth
    most where the attention stream's PV matmuls start waiting on exp."""
    if not lb:
        return list(fill)
    n3 = len(fill)
    f1, f2 = n3 // 4, n3 // 2
    b1, b2 = len(lb) // 3, 2 * len(lb) // 3
    return (_weighted_merge(fill[:f1], lb[:b1])
            + _weighted_merge(fill[f1:f2], lb[b1:b2])
            + _weighted_merge(fill[f2:], lb[b2:]))


def _build():
    nc = bass.Bass("TRN2", target_bir_lowering=False, debug=False,
                   num_devices=NCORE)

    # host pre-chunks everything into the exact SBUF layouts so every DMA
    # reads fully contiguous DRAM (big bursts, few descriptors)
    xqT = nc.dram_tensor("xqT", [NT, 128, KPC * TW], fmm, kind="ExternalInput").ap()
    xkT = nc.dram_tensor("xkT", [NT, 128, KPC * TW], fmm, kind="ExternalInput").ap()
    xvT = nc.dram_tensor("xvT", [NT, 128, KPC * TW], fmm, kind="ExternalInput").ap()
    wq_d = nc.dram_tensor("wq", [128, KPC * DOUT], fmm, kind="ExternalInput").ap()
    wk_d = nc.dram_tensor("wk", [128, KPC * DOUT], fmm, kind="ExternalInput").ap()
    wv_d = nc.dram_tensor("wv", [128, KPC * DOUT], fmm, kind="ExternalInput").ap()
    wo_d = nc.dram_tensor("wo", [128, 2 * D], fmm, kind="ExternalInput").ap()
    bq_d = nc.dram_tensor("bq", [DOUT, 1], f32, kind="ExternalInput").ap()
    bk_d = nc.dram_tensor("bk", [DOUT, 1], f32, kind="ExternalInput").ap()
    tri_d = nc.dram_tensor("tri", [128, 128], fmm, kind="ExternalInput").ap()
    out_d = nc.dram_tensor("out", [S, D], fmm, kind="ExternalOutput").ap()

    with tile.TileContext(nc) as tc:
        with (
            tc.tile_pool(name="const", bufs=1) as cpool,
            tc.tile_pool(name="qk", bufs=1) as qkpool,
            tc.tile_pool(name="vo", bufs=1) as vopool,
            tc.tile_pool(name="xt", bufs=8) as xtpool,
            tc.tile_pool(name="pexp", bufs=8) as pepool,
            tc.tile_pool(name="rec", bufs=3) as recpool,
            tc.tile_pool(name="ctx", bufs=6) as ctxpool,
            tc.tile_pool(name="ost", bufs=3) as ostpool,
            tc.tile_pool(name="pp", bufs=2, space="PSUM") as pppool,
            tc.tile_pool(name="psc", bufs=2, space="PSUM") as scpool,
            tc.tile_pool(name="pcx", bufs=1, space="PSUM") as cxpool,
        ):
            wq_t = cpool.tile([128, KPC * DOUT], fmm, tag="wq")
            wk_t = cpool.tile([128, KPC * DOUT], fmm, tag="wk")
            wv_t = cpool.tile([128, KPC * DOUT], fmm, tag="wv")
            wo_t = cpool.tile([128, 2 * D], fmm, tag="wo")
            bq_t = cpool.tile([128, 2], f32, tag="bq")
            bk_t = cpool.tile([128, 2], f32, tag="bk")
            tri_t = cpool.tile([128, 128], fmm, tag="tri")
            warm_t = cpool.tile([128, TW], fmm, tag="warm")

            # One [V | ones] tensor, 512 cols per k-block: head h of block kb
            # at cols (kb*HPC+h)*128 (V written by the V projection, ones by
            # the memset below).
            vones_t = vopool.tile([128, NKB * HPC * 128], fmm, tag="vones",
                                  name="vones")
            vones = [vones_t[:, kb * HPC * 128:(kb + 1) * HPC * 128]
                     for kb in range(NKB)]

            # ---- PE clock pre-warm: ~4.3us of dummy matmuls issued while
            # the DMA rings spin up, so HAM reaches K=8/8 (2.4 GHz) right as
            # the first projection's inputs land. The dummy PSUM tile lives
            # in the scores pool, which is idle until attention starts. ----
            nc.vector.memset(warm_t[:], 0.0)
            wpsum = scpool.tile([128, 2 * TW], f32, tag="sc", name="warm")

            def dummy_mm(n=1):
                for _ in range(n):
                    nc.tensor.matmul(wpsum[:, 0:TW], warm_t[:, 0:128],
                                     warm_t[:], start=True, stop=True)

            dummy_mm(NWARM)
            # pull the ~2.7us exp/ln ACT table load into the idle startup
            nc.scalar.activation(warm_t[:, 0:1], warm_t[:, 1:2], EXP)
            nc.scalar.activation(warm_t[:, 0:1], warm_t[:, 1:2], LN)
            nc.vector.memset(
                vones_t[:].rearrange("p (s c) -> p s c",
                                     s=NKB * HPC)[:, :, HD:128], 1.0)

            qt = [[None] * NT for _ in range(2)]
            kt = [[None] * NT for _ in range(2)]
            ctx_chunks = [[None] * 2 for _ in range(NT)]
            xts = {}

            def dma_task(t):
                def f():
                    if t == 0:
                        # EVERYTHING rides the sync ring in strict priority
                        # order: the 16 SDMA engines round-robin rings at
                        # packet granularity, so a second ring's big
                        # descriptors would starve the small urgent chunks
                        # here (measured: xq chunks crawled 12->21us when
                        # xk/xv ran on other rings).
                        xxq = xtpool.tile([128, KPC * TW], fmm, tag="xt",
                                          name="xt_q_0")
                        xxk = xtpool.tile([128, KPC * TW], fmm, tag="xt",
                                          name="xt_k_0")
                        xxv = xtpool.tile([128, KPC * TW], fmm, tag="xt",
                                          name="xt_v_0")
                        nc.sync.dma_start(wq_t[:, 0:DOUT], wq_d[:, 0:DOUT])
                        nc.sync.dma_start(xxq[:, 0:TW], xqT[0, :, 0:TW])
                        nc.sync.dma_start(
                            bq_t[:], bq_d.rearrange("(mc p) o -> p (mc o)",
                                                    p=128))
                        nc.sync.dma_start(
                            bk_t[:], bk_d.rearrange("(mc p) o -> p (mc o)",
                                                    p=128))
                        nc.sync.dma_start(xxq[:, TW:2 * TW],
                                          xqT[0, :, TW:2 * TW])
                        nc.sync.dma_start(wq_t[:, DOUT:], wq_d[:, DOUT:])
                        nc.sync.dma_start(xxq[:, 2 * TW:],
                                          xqT[0, :, 2 * TW:])
                        nc.sync.dma_start(wv_t[:], wv_d[:])
                        nc.sync.dma_start(wk_t[:], wk_d[:])
                        nc.sync.dma_start(xxk[:], xkT[0])
                        nc.sync.dma_start(xxv[:], xvT[0])
                        nc.sync.dma_start(tri_t[:], tri_d[:])
                        nc.sync.dma_start(wo_t[:], wo_d[:])
                        xts[("q", 0)] = xxq
                        xts[("k", 0)] = xxk
                        xts[("v", 0)] = xxv
                    else:
                        for name, x_d in (("q", xqT), ("k", xkT),
                                          ("v", xvT)):
                            xx = xtpool.tile([128, KPC * TW], fmm, tag="xt",
                                             name=f"xt_{name}_{t}")
                            nc.sync.dma_start(xx[:], x_d[t])
                            xts[(name, t)] = xx
                return f

            def qk_task(name, w_t, b_t, dst, mc, t):
                psum = pppool.tile([128, TW], f32, tag="pp",
                                   name=f"pp_{name}{mc}_{t}")
                for kc in range(KPC):
                    nc.tensor.matmul(
                        psum[:],
                        w_t[:, kc * DOUT + mc * 128:
                            kc * DOUT + (mc + 1) * 128],
                        xts[(name, t)][:, kc * TW:(kc + 1) * TW],
                        start=(kc == 0), stop=(kc == KPC - 1),
                    )
                piece = qkpool.tile([128, TW], fmm, tag=f"{name}t{mc}{t}",
                                    name=f"{name}t{mc}{t}")
                nc.vector.tensor_scalar_add(piece[:], psum[:],
                                            b_t[:, mc:mc + 1])
                dst[mc][t] = piece

            def v_task(sc, t):
                kb = t * 4 + sc
                psv = pppool.tile([128, DOUT], f32, tag="pp",
                                  name=f"ppv{sc}_{t}")
                for kc in range(KPC):
                    nc.tensor.matmul(
                        psv[:],
                        xts[("v", t)][:, kc * TW + sc * 128:
                                      kc * TW + (sc + 1) * 128],
                        wv_t[:, kc * DOUT:(kc + 1) * DOUT],
                        start=(kc == 0), stop=(kc == KPC - 1),
                    )
                # one strided copy drops all 4 heads into their V slots
                nc.vector.tensor_copy(
                    vones[kb].rearrange("p (s c) -> p s c",
                                        s=HPC)[:, :, 0:HD],
                    psv[:].rearrange("p (h d) -> p h d", h=HPC))

            def a_mm(t):
                tasks = []
                if t == 0:
                    # q first (its chunks land first), then dummies to cover
                    # the known wait for wk/xk, then k, then v (xv arrives
                    # while the k projections run). The dummies must sit in
                    # the PE queue BEFORE the gated matmul — the queue is
                    # in-order, filler behind a stalled op never runs.
                    tasks.append(lambda: qk_task("q", wq_t, bq_t, qt, 0, 0))
                    tasks.append(lambda: qk_task("q", wq_t, bq_t, qt, 1, 0))
                    tasks.append(lambda: dummy_mm(12))
                    tasks.append(lambda: qk_task("k", wk_t, bk_t, kt, 0, 0))
                    tasks.append(lambda: qk_task("k", wk_t, bk_t, kt, 1, 0))
                    tasks.append(lambda: dummy_mm(6))
                    for sc in range(4):
                        tasks.append(lambda sc=sc: v_task(sc, 0))
                    return tasks
                for mc in range(2):
                    tasks.append(lambda mc=mc, t=t: qk_task("q", wq_t, bq_t,
                                                            qt, mc, t))
                    tasks.append(lambda mc=mc, t=t: qk_task("k", wk_t, bk_t,
                                                            kt, mc, t))
                for sc in range(4):
                    tasks.append(lambda sc=sc, t=t: v_task(sc, t))
                return tasks

            def b_stream(t, hp):
                cxt = {}
                pets = {}

                def alloc(t=t, hp=hp):
                    cxt["tile"] = cxpool.tile([128, 2 * TW], f32, tag="cx",
                                              name=f"cx{hp}_{t}")

                def s_task(kb, t=t, hp=hp):
                    sub = max(0, (kb - 4 * t) * 128)
                    spsum = scpool.tile([128, 2 * TW], f32, tag="sc",
                                        name=f"sc{hp}{kb}_{t}")
                    for hi in range(2):
                        nc.tensor.matmul(
                            spsum[:, hi * TW + sub:(hi + 1) * TW],
                            kt[hp][kb // 4][hi * HD:(hi + 1) * HD,
                                            (kb % 4) * 128:
                                            (kb % 4 + 1) * 128],
                            qt[hp][t][hi * HD:(hi + 1) * HD, sub:TW],
                            start=True, stop=True,
                        )
                    pet = pepool.tile([128, 2 * TW], fmm, tag="pex",
                                      name=f"pex{hp}{kb}_{t}")
                    pv = spsum[:].rearrange("p (h c) -> p h c", h=2)
                    ev = pet[:].rearrange("p (h c) -> p h c", h=2)
                    nc.scalar.activation(ev[:, :, sub:TW], pv[:, :, sub:TW],
                                         EXP, scale=0.125)
                    if kb >= 4 * t:  # diagonal block: mask the triangle
                        for hi in range(2):
                            seg = pet[:, hi * TW + sub:hi * TW + sub + 128]
                            nc.vector.tensor_mul(seg, seg, tri_t[:])
                    pets[kb] = (pet, sub)

                def p_task(kb, t=t, hp=hp):
                    pet, sub = pets.pop(kb)
                    cpsum = cxt["tile"]
                    for hi in range(2):
                        h = 2 * hp + hi
                        nc.tensor.matmul(
                            cpsum[:, hi * TW + sub:(hi + 1) * TW],
                            vones[kb][:, h * 128:(h + 1) * 128],
                            pet[:, hi * TW + sub:(hi + 1) * TW],
                            start=(kb == 0), stop=(kb == 4 * t + 3),
                        )

                def norm(t=t, hp=hp):
                    cpsum = cxt["tile"]
                    rec = recpool.tile([HD, 2 * TW], f32, tag="rec",
                                       name=f"rec{hp}_{t}")
                    ltmp = recpool.tile([HD, 2 * TW], f32, tag="ltmp",
                                        name=f"ltmp{hp}_{t}")
                    cchunk = ctxpool.tile([128, TW], fmm, tag="cc",
                                          name=f"cc{hp}_{t}")
                    # for the last tile, emit reciprocal + normalize per
                    # q-quarter so the final out-projections unblock
                    # progressively instead of behind one 2.2us recip
                    nq = 4 if t == NT - 1 else 1
                    qw = TW // nq
                    sv = cpsum[HD:128, :].rearrange("p (h c) -> p h c", h=2)
                    rv = rec[:].rearrange("p (h c) -> p h c", h=2)
                    lv = ltmp[:].rearrange("p (h c) -> p h c", h=2)
                    for q in range(nq):
                        qs = slice(q * qw, (q + 1) * qw)
                        _act_recip(nc, rv[:, :, qs], sv[:, :, qs],
                                   lv[:, :, qs])
                        for hi in range(2):
                            nc.vector.tensor_mul(
                                cchunk[hi * HD:(hi + 1) * HD,
                                       q * qw:(q + 1) * qw],
                                cpsum[0:HD, hi * TW + q * qw:
                                      hi * TW + (q + 1) * qw],
                                rec[:, hi * TW + q * qw:
                                    hi * TW + (q + 1) * qw])
                    ctx_chunks[t][hp] = cchunk

                nkb = 4 * t + 4
                tasks = [alloc, lambda: s_task(0)]
                for k in range(1, nkb):
                    tasks.append(lambda k=k: s_task(k))
                    tasks.append(lambda k=k: p_task(k - 1))
                tasks.append(lambda: p_task(nkb - 1))
                tasks.append(norm)
                return tasks

            def o_task(qc, t):
                ost = ostpool.tile([128, D], fmm, tag="ost",
                                   name=f"ost{qc}_{t}")
                for on in range(2):
                    pso = pppool.tile([128, TW], f32, tag="pp",
                                      name=f"po{qc}{on}_{t}")
                    for hc in range(2):
                        nc.tensor.matmul(
                            pso[:],
                            ctx_chunks[t][hc][:, qc * 128:(qc + 1) * 128],
                            wo_t[:, hc * D + on * TW:
                                 hc * D + (on + 1) * TW],
                            start=(hc == 0), stop=(hc == 1),
                        )
                    # for the last tile split the PSUM->fp16 casts across
                    # DVE and ACT (both idle by then) to shorten the drain
                    if t == NT - 1 and on == 1:
                        nc.scalar.copy(ost[:, on * TW:(on + 1) * TW],
                                       pso[:])
                    else:
                        nc.vector.tensor_copy(ost[:, on * TW:(on + 1) * TW],
                                              pso[:])
                    # outputs alternate between the ACT ring and the sync
                    # ring (idle between prefetches) so the final drain
                    # isn't serialized on one ring
                    row = t * TW + qc * 128
                    eng = nc.scalar if (qc + on) % 2 else nc.sync
                    eng.dma_start(
                        out_d[row:row + 128, on * TW:(on + 1) * TW],
                        ost[:, on * TW:(on + 1) * TW])

            def c_list(t):
                return [lambda qc=qc, t=t: o_task(qc, t) for qc in range(4)]

            bs = {}
            for t in range(NT):
                for hp in range(2):
                    bs[(t, hp)] = b_stream(t, hp)
            # tile-3/hp0 split: [alloc, S0, (S1,P0)..(S11,P10)] goes a phase
            # early; [S12..S15, P11..P15, norm] stays.
            cut = 2 + 2 * 11
            phases = [
                ([dma_task(0)] + a_mm(0) + [dma_task(1)], [], []),
                (a_mm(1) + [dma_task(2)], bs[(0, 0)] + bs[(0, 1)], []),
                (a_mm(2) + [dma_task(3)], bs[(1, 0)] + bs[(1, 1)], c_list(0)),
                (a_mm(3), bs[(2, 0)] + bs[(2, 1)] + bs[(3, 0)][:cut],
                 c_list(1)),
                ([], bs[(3, 0)][cut:] + bs[(3, 1)], c_list(2)),
                ([], [], c_list(3)),
            ]
            for la, lb, lc in phases:
                for task in _tail_merge(lb, _weighted_merge(lc, la)):
                    task()

    _split_sync_waits(nc)
    return nc


_NC = None
TRACE = False
LAST_RESULTS = None


def kernel(query, key, value, attn_mask, Wq, bq, Wk, bk, Wv, bv, Wo, bo):
    global _NC, LAST_RESULTS
    query = np.asarray(query, np.float32)
    key = np.asarray(key, np.float32)
    value = np.asarray(value, np.float32)
    attn_mask = np.asarray(attn_mask, np.float32)
    Wq, Wk, Wv, Wo = (np.asarray(w, np.float32) for w in (Wq, Wk, Wv, Wo))
    bq, bk, bv, bo = (np.asarray(b, np.float32) for b in (bq, bk, bv, bo))

    if _NC is None:
        _NC = _build()

    hdt = np.float16
    # S^T tile element (i, j): keep k-row i iff attn_mask[q=j, k=i] == 0
    tri = np.ascontiguousarray((attn_mask[:128, :128].T == 0).astype(hdt))

    def chunk_x(x):
        # [S, D] -> xT [D, S] -> [NT, 128, KPC*TW]: out[t, p, kc*TW+c] =
        # x[t*TW+c, kc*128+p]
        xt = x.T.astype(hdt).reshape(KPC, 128, NT, TW)
        return np.ascontiguousarray(xt.transpose(2, 1, 0, 3)).reshape(
            NT, 128, KPC * TW)

    def chunk_w(w):
        # [D, DOUT] -> [128, KPC*DOUT]: out[p, kc*DOUT+m] = w[kc*128+p, m]
        return np.ascontiguousarray(
            w.astype(hdt).reshape(KPC, 128, DOUT).transpose(1, 0, 2)).reshape(
            128, KPC * DOUT)

    def chunk_wo(w):
        # [DOUT, D] -> [128, 2*D]
        return np.ascontiguousarray(
            w.astype(hdt).reshape(2, 128, D).transpose(1, 0, 2)).reshape(
            128, 2 * D)

    xT = {}
    for b in range(B):
        xT[("q", b)] = chunk_x(query[b])
        xT[("k", b)] = chunk_x(key[b])
        xT[("v", b)] = chunk_x(value[b])

    in_maps = []
    for c in range(NCORE):
        b, g = divmod(c, NCORE // B)
        sl = slice(g * DOUT, (g + 1) * DOUT)
        in_maps.append({
            "xqT": xT[("q", b)], "xkT": xT[("k", b)], "xvT": xT[("v", b)],
            "wq": chunk_w(Wq[:, sl]),
            "wk": chunk_w(Wk[:, sl]),
            "wv": chunk_w(Wv[:, sl]),
            "wo": chunk_wo(Wo[sl, :]),
            "bq": np.ascontiguousarray(bq[sl])[:, None],
            "bk": np.ascontiguousarray(bk[sl])[:, None],
            "tri": tri,
        })

    res = run_bass_kernel_spmd(_NC, in_maps, core_ids=list(range(NCORE)),
                               trace=TRACE)
    LAST_RESULTS = res

    extra = (bv @ Wo + bo).astype(np.float32)
    out = np.empty((B, S, D), np.float32)
    for b in range(B):
        acc = res.results[b * 4]["out"].astype(np.float32).copy()
        for g in range(1, NCORE // B):
            acc += res.results[b * 4 + g]["out"]
        out[b] = acc + extra
    return out


# revision 22
# speedup vs baseline: 1.0310x; 1.0310x over previous
"""Multi-head attention (B=2, S=2048, D=1024, H=16, causal) on 8 TRN2 cores.

Sharding: batch (2) x head-groups (4 heads per core). Each core:
  - projects its 4 heads' Q/K/V (fp16 matmuls, full PE rate)
  - causal flash attention in transposed layout:
      S^T[k,q] = Kt.T @ Qt  (K=64 contraction; the two heads of a pair are
            emitted as row-tiled matmuls at base partitions 0/64 so the PE
            runs them concurrently in one 512-cycle slot)
      P^T = exp(S^T/8) via ACT straight from PSUM; diagonal blocks masked in
            place with a 0/1 triangle multiply on DVE
      ctx^T+sumexp = [V | ones].T @ P^T accumulated over k-blocks in PSUM;
            the 64 ones-columns replicate sumexp across partitions (free in
            PE time: matmul cost is N cycles regardless of M) so the
            normalize is reciprocal (ACT ln+exp) + plain multiplies (DVE)
  - partial out-projection out_c = ctx_norm^T.T @ Wo[slice]
Host: out[b] = sum over the batch's 4 cores + bo + bv @ Wo.

Schedule details (all measured off perfetto traces):
  - ~10 dummy matmuls on a memset scratch tile run during the DMA spin-up so
    the PE_HAM clock gate un-throttles (1.2 -> 2.4 GHz) before real work.
  - weight/x DMAs are chunked and ordered so the first projection's
    dependencies land ~1us after the HWDGE ring starts; wv/xk/xv/tri/wo ride
    the ACT ring in parallel; outputs ride the idle gpsimd SWDGE ring.
  - the ones-columns are written by a single strided DVE memset (the DMA
    version costs ~8k descriptors on the sync ring).
  - per head-pair the scores matmul for k-block kb+1 is emitted BEFORE the
    PV matmul of kb: PV waits on the ACT exp, and the PE queue is in-order,
    so this keeps a score slot runnable while exp catches up.
  - tile-3's first head-pair attention is pulled one phase earlier to spread
    the terminal ACT(exp)-heavy stretch across more PE work.
"""
import sys

sys.path.insert(0, "/opt/trn_rl_repo")

import numpy as np
import concourse.bass as bass
import concourse.tile as tile
import concourse.mybir as mybir
from concourse.bass_utils import run_bass_kernel_spmd
B, S, D, NH, HD = 2, 2048, 1024, 16, 64
NCORE = 8
HPC = NH // (NCORE // B)      # heads per core = 4
DOUT = HPC * HD               # 256 per-core projection width
NT = 4                        # seq tiles of 512
TW = S // NT                  # 512
NKB = S // 128                # 16 k-blocks
KPC = D // 128                # 8 contraction chunks for projections
NWARM = 8

f32 = mybir.dt.float32
# fp16 (10-bit mantissa) streams 1 row/cycle on the PE and gets Fast Weight
# Load; fp32r needs 2 half-rate passes. End-to-end error stays ~5e-4.
fmm = mybir.dt.float16
EXP = mybir.ActivationFunctionType.Exp
LN = mybir.ActivationFunctionType.Ln


def _split_sync_waits(nc):
    """walrus rejects >1 sync wait on most instructions; hoist extras onto
    preceding NoOps on the same engine (sems are monotone, so waiting
    earlier is always safe)."""
    for func in nc.m.functions:
        for blk in func.blocks:
            insts = list(blk.instructions)
            out = []
            changed = False
            for inst in insts:
                si = inst.sync_info
                waits = list(si.on_wait) if (si is not None and si.on_wait) else []
                if len(waits) > 1:
                    hoist, keep = waits[:-1], waits[-1:]
                    for i, w in enumerate(hoist):
                        nop = mybir.InstNoOp(
                            name=f"{inst.name}-ws{i}",
                            engine=inst.engine,
                            sync_info=mybir.SyncInfo(on_wait=[w], on_update=[]),
                        )
                        nop.bass_nofuse = True
                        out.append(nop)
                    inst.sync_info = mybir.SyncInfo(
                        on_wait=keep, on_update=list(si.on_update)
                    )
                    changed = True
                out.append(inst)
            if changed:
                blk.instructions = out


def _act_recip(nc, out, in_, tmp):
    # 1/x = exp(-ln(x)). Ln and Exp share one ACT table set
    # (natural_log_exp_and_others), so this costs two streaming passes and
    # zero table reloads.
    nc.scalar.activation(tmp, in_, LN)
    nc.scalar.activation(out, tmp, EXP, scale=-1.0)


def _weighted_merge(la, lb):
    out = []
    ia = ib = 0
    na, nb = len(la), len(lb)
    while ia < na or ib < nb:
        if ib >= nb or (ia < na and ia * nb <= ib * na):
            out.append(la[ia]); ia += 1
        else:
            out.append(lb[ib]); ib += 1
    return out


def _tail_merge(lb, fill):
    """Merge filler into the b-stream with density increasing toward the
    end: ACT (exp) lag accumulates over a phase, so PE-ready filler is worth
    most where the attention stream's PV matmuls start waiting on exp."""
    if not lb:
        return list(fill)
    n3 = len(fill)
    f1, f2 = n3 // 4, n3 // 2
    b1, b2 = len(lb) // 3, 2 * len(lb) // 3
    return (_weighted_merge(fill[:f1], lb[:b1])
            + _weighted_merge(fill[f1:f2], lb[b1:b2])
            + _weighted_merge(fill[f2:], lb[b2:]))


def _build():
    nc = bass.Bass("TRN2", target_bir_lowering=False, debug=False,
                   num_devices=NCORE)

    # host pre-chunks everything into the exact SBUF layouts so every DMA
    # reads fully contiguous DRAM (big bursts, few descriptors)
    xqT = nc.dram_tensor("xqT", [NT, 128, KPC * TW], fmm, kind="ExternalInput").ap()
    xkT = nc.dram_tensor("xkT", [NT, 128, KPC * TW], fmm, kind="ExternalInput").ap()
    xvT = nc.dram_tensor("xvT", [NT, 128, KPC * TW], fmm, kind="ExternalInput").ap()
    wq_d = nc.dram_tensor("wq", [128, KPC * DOUT], fmm, kind="ExternalInput").ap()
    wk_d = nc.dram_tensor("wk", [128, KPC * DOUT], fmm, kind="ExternalInput").ap()
    wv_d = nc.dram_tensor("wv", [128, KPC * DOUT], fmm, kind="ExternalInput").ap()
    wo_d = nc.dram_tensor("wo", [128, 2 * D], fmm, kind="ExternalInput").ap()
    bq_d = nc.dram_tensor("bq", [DOUT, 1], f32, kind="ExternalInput").ap()
    bk_d = nc.dram_tensor("bk", [DOUT, 1], f32, kind="ExternalInput").ap()
    tri_d = nc.dram_tensor("tri", [128, 128], fmm, kind="ExternalInput").ap()
    out_d = nc.dram_tensor("out", [S, D], fmm, kind="ExternalOutput").ap()

    with tile.TileContext(nc) as tc:
        with (
            tc.tile_pool(name="const", bufs=1) as cpool,
            tc.tile_pool(name="qk", bufs=1) as qkpool,
            tc.tile_pool(name="vo", bufs=1) as vopool,
            tc.tile_pool(name="xt", bufs=8) as xtpool,
            tc.tile_pool(name="pexp", bufs=8) as pepool,
            tc.tile_pool(name="rec", bufs=3) as recpool,
            tc.tile_pool(name="ctx", bufs=6) as ctxpool,
            tc.tile_pool(name="ost", bufs=3) as ostpool,
            tc.tile_pool(name="pp", bufs=2, space="PSUM") as pppool,
            tc.tile_pool(name="psc", bufs=2, space="PSUM") as scpool,
            tc.tile_pool(name="pcx", bufs=1, space="PSUM") as cxpool,
        ):
            wq_t = cpool.tile([128, KPC * DOUT], fmm, tag="wq")
            wk_t = cpool.tile([128, KPC * DOUT], fmm, tag="wk")
            wv_t = cpool.tile([128, KPC * DOUT], fmm, tag="wv")
            wo_t = cpool.tile([128, 2 * D], fmm, tag="wo")
            bq_t = cpool.tile([128, 2], f32, tag="bq")
            bk_t = cpool.tile([128, 2], f32, tag="bk")
            tri_t = cpool.tile([128, 128], fmm, tag="tri")
            warm_t = cpool.tile([128, TW], fmm, tag="warm")

            # One [V | ones] tensor, 512 cols per k-block: head h of block kb
            # at cols (kb*HPC+h)*128 (V written by the V projection, ones by
            # the memset below).
            vones_t = vopool.tile([128, NKB * HPC * 128], fmm, tag="vones",
                                  name="vones")
            vones = [vones_t[:, kb * HPC * 128:(kb + 1) * HPC * 128]
                     for kb in range(NKB)]

            # ---- PE clock pre-warm: ~4.3us of dummy matmuls issued while
            # the DMA rings spin up, so HAM reaches K=8/8 (2.4 GHz) right as
            # the first projection's inputs land. The dummy PSUM tile lives
            # in the scores pool, which is idle until attention starts. ----
            nc.vector.memset(warm_t[:], 0.0)
            wpsum = scpool.tile([128, 2 * TW], f32, tag="sc", name="warm")

            def dummy_mm(n=1):
                for _ in range(n):
                    nc.tensor.matmul(wpsum[:, 0:TW], warm_t[:, 0:128],
                                     warm_t[:], start=True, stop=True)

            dummy_mm(NWARM)
            # pull the ~2.7us exp/ln ACT table load into the idle startup
            nc.scalar.activation(warm_t[:, 0:1], warm_t[:, 1:2], EXP)
            nc.scalar.activation(warm_t[:, 0:1], warm_t[:, 1:2], LN)
            nc.vector.memset(
                vones_t[:].rearrange("p (s c) -> p s c",
                                     s=NKB * HPC)[:, :, HD:128], 1.0)

            qt = [[None] * NT for _ in range(2)]
            kt = [[None] * NT for _ in range(2)]
            ctx_chunks = [[None] * 2 for _ in range(NT)]
            xts = {}

            def dma_task(t):
                def f():
                    if t == 0:
                        # EVERYTHING rides the sync ring in strict priority
                        # order: the 16 SDMA engines round-robin rings at
                        # packet granularity, so a second ring's big
                        # descriptors would starve the small urgent chunks
                        # here (measured: xq chunks crawled 12->21us when
                        # xk/xv ran on other rings).
                        xxq = xtpool.tile([128, KPC * TW], fmm, tag="xt",
                                          name="xt_q_0")
                        xxk = xtpool.tile([128, KPC * TW], fmm, tag="xt",
                                          name="xt_k_0")
                        xxv = xtpool.tile([128, KPC * TW], fmm, tag="xt",
                                          name="xt_v_0")
                        nc.sync.dma_start(wq_t[:, 0:DOUT], wq_d[:, 0:DOUT])
                        nc.sync.dma_start(xxq[:, 0:TW], xqT[0, :, 0:TW])
                        nc.sync.dma_start(
                            bq_t[:], bq_d.rearrange("(mc p) o -> p (mc o)",
                                                    p=128))
                        nc.sync.dma_start(
                            bk_t[:], bk_d.rearrange("(mc p) o -> p (mc o)",
                                                    p=128))
                        nc.sync.dma_start(xxq[:, TW:2 * TW],
                                          xqT[0, :, TW:2 * TW])
                        nc.sync.dma_start(wq_t[:, DOUT:], wq_d[:, DOUT:])
                        nc.sync.dma_start(xxq[:, 2 * TW:],
                                          xqT[0, :, 2 * TW:])
                        nc.sync.dma_start(wv_t[:], wv_d[:])
                        nc.sync.dma_start(wk_t[:], wk_d[:])
                        nc.sync.dma_start(xxk[:], xkT[0])
                        nc.sync.dma_start(xxv[:], xvT[0])
                        nc.sync.dma_start(tri_t[:], tri_d[:])
                        nc.sync.dma_start(wo_t[:], wo_d[:])
                        xts[("q", 0)] = xxq
                        xts[("k", 0)] = xxk
                        xts[("v", 0)] = xxv
                    else:
                        for name, x_d in (("q", xqT), ("k", xkT),
                                          ("v", xvT)):
                            xx = xtpool.tile([128, KPC * TW], fmm, tag="xt",
                                             name=f"xt_{name}_{t}")
                            nc.sync.dma_start(xx[:], x_d[t])
                            xts[(name, t)] = xx
                return f

            def qk_task(name, w_t, b_t, dst, mc, t):
                psum = pppool.tile([128, TW], f32, tag="pp",
                                   name=f"pp_{name}{mc}_{t}")
                for kc in range(KPC):
                    nc.tensor.matmul(
                        psum[:],
                        w_t[:, kc * DOUT + mc * 128:
                            kc * DOUT + (mc + 1) * 128],
                        xts[(name, t)][:, kc * TW:(kc + 1) * TW],
                        start=(kc == 0), stop=(kc == KPC - 1),
                    )
                piece = qkpool.tile([128, TW], fmm, tag=f"{name}t{mc}{t}",
                                    name=f"{name}t{mc}{t}")
                nc.vector.tensor_scalar_add(piece[:], psum[:],
                                            b_t[:, mc:mc + 1])
                dst[mc][t] = piece

            def v_task(sc, t):
                kb = t * 4 + sc
                psv = pppool.tile([128, DOUT], f32, tag="pp",
                                  name=f"ppv{sc}_{t}")
                for kc in range(KPC):
                    nc.tensor.matmul(
                        psv[:],
                        xts[("v", t)][:, kc * TW + sc * 128:
                                      kc * TW + (sc + 1) * 128],
                        wv_t[:, kc * DOUT:(kc + 1) * DOUT],
                        start=(kc == 0), stop=(kc == KPC - 1),
                    )
                # one strided copy drops all 4 heads into their V slots
                nc.vector.tensor_copy(
                    vones[kb].rearrange("p (s c) -> p s c",
                                        s=HPC)[:, :, 0:HD],
                    psv[:].rearrange("p (h d) -> p h d", h=HPC))

            def a_mm(t):
                tasks = []
                if t == 0:
                    # q first (its chunks land first), then dummies to cover
                    # the known wait for wk/xk, then k, then v (xv arrives
                    # while the k projections run). The dummies must sit in
                    # the PE queue BEFORE the gated matmul — the queue is
                    # in-order, filler behind a stalled op never runs.
                    tasks.append(lambda: qk_task("q", wq_t, bq_t, qt, 0, 0))
                    tasks.append(lambda: qk_task("q", wq_t, bq_t, qt, 1, 0))
                    tasks.append(lambda: dummy_mm(12))
                    tasks.append(lambda: qk_task("k", wk_t, bk_t, kt, 0, 0))
                    tasks.append(lambda: qk_task("k", wk_t, bk_t, kt, 1, 0))
                    tasks.append(lambda: dummy_mm(6))
                    for sc in range(4):
                        tasks.append(lambda sc=sc: v_task(sc, 0))
                    return tasks
                for mc in range(2):
                    tasks.append(lambda mc=mc, t=t: qk_task("q", wq_t, bq_t,
                                                            qt, mc, t))
                    tasks.append(lambda mc=mc, t=t: qk_task("k", wk_t, bk_t,
                                                            kt, mc, t))
                for sc in range(4):
                    tasks.append(lambda sc=sc, t=t: v_task(sc, t))
                return tasks

            def b_stream(t, hp):
                cxt = {}
                pets = {}

                def alloc(t=t, hp=hp):
                    cxt["tile"] = cxpool.tile([128, 2 * TW], f32, tag="cx",
                                              name=f"cx{hp}_{t}")

                def s_task(kb, t=t, hp=hp):
                    sub = max(0, (kb - 4 * t) * 128)
                    spsum = scpool.tile([128, 2 * TW], f32, tag="sc",
                                        name=f"sc{hp}{kb}_{t}")
                    for hi in range(2):
                        nc.tensor.matmul(
                            spsum[:, hi * TW + sub:(hi + 1) * TW],
                            kt[hp][kb // 4][hi * HD:(hi + 1) * HD,
                                            (kb % 4) * 128:
                                            (kb % 4 + 1) * 128],
                            qt[hp][t][hi * HD:(hi + 1) * HD, sub:TW],
                            start=True, stop=True,
                        )
                    pet = pepool.tile([128, 2 * TW], fmm, tag="pex",
                                      name=f"pex{hp}{kb}_{t}")
                    pv = spsum[:].rearrange("p (h c) -> p h c", h=2)
                    ev = pet[:].rearrange("p (h c) -> p h c", h=2)
                    nc.scalar.activation(ev[:, :, sub:TW], pv[:, :, sub:TW],
                                         EXP, scale=0.125)
                    if kb >= 4 * t:  # diagonal block: mask the triangle
                        for hi in range(2):
                            seg = pet[:, hi * TW + sub:hi * TW + sub + 128]
                            nc.vector.tensor_mul(seg, seg, tri_t[:])
                    pets[kb] = (pet, sub)

                def p_task(kb, t=t, hp=hp):
                    pet, sub = pets.pop(kb)
                    cpsum = cxt["tile"]
                    for hi in range(2):
                        h = 2 * hp + hi
                        nc.tensor.matmul(
                            cpsum[:, hi * TW + sub:(hi + 1) * TW],
                            vones[kb][:, h * 128:(h + 1) * 128],
                            pet[:, hi * TW + sub:(hi + 1) * TW],
                            start=(kb == 0), stop=(kb == 4 * t + 3),
                        )

                def norm(t=t, hp=hp):
                    cpsum = cxt["tile"]
                    rec = recpool.tile([HD, 2 * TW], f32, tag="rec",
                                       name=f"rec{hp}_{t}")
                    ltmp = recpool.tile([HD, 2 * TW], f32, tag="ltmp",
                                        name=f"ltmp{hp}_{t}")
                    cchunk = ctxpool.tile([128, TW], fmm, tag="cc",
                                          name=f"cc{hp}_{t}")
                    # for the last tile, emit reciprocal + normalize per
                    # q-quarter so the final out-projections unblock
                    # progressively instead of behind one 2.2us recip
                    nq = 4 if t == NT - 1 else 1
                    qw = TW // nq
                    sv = cpsum[HD:128, :].rearrange("p (h c) -> p h c", h=2)
                    rv = rec[:].rearrange("p (h c) -> p h c", h=2)
                    lv = ltmp[:].rearrange("p (h c) -> p h c", h=2)
                    for q in range(nq):
                        qs = slice(q * qw, (q + 1) * qw)
                        _act_recip(nc, rv[:, :, qs], sv[:, :, qs],
                                   lv[:, :, qs])
                        for hi in range(2):
                            nc.vector.tensor_mul(
                                cchunk[hi * HD:(hi + 1) * HD,
                                       q * qw:(q + 1) * qw],
                                cpsum[0:HD, hi * TW + q * qw:
                                      hi * TW + (q + 1) * qw],
                                rec[:, hi * TW + q * qw:
                                    hi * TW + (q + 1) * qw])
                    ctx_chunks[t][hp] = cchunk

                nkb = 4 * t + 4
                tasks = [alloc, lambda: s_task(0)]
                for k in range(1, nkb):
                    tasks.append(lambda k=k: s_task(k))
                    tasks.append(lambda k=k: p_task(k - 1))
                tasks.append(lambda: p_task(nkb - 1))
                tasks.append(norm)
                return tasks

            def o_task(qc, t):
                ost = ostpool.tile([128, D], fmm, tag="ost",
                                   name=f"ost{qc}_{t}")
                for on in range(2):
                    pso = pppool.tile([128, TW], f32, tag="pp",
                                      name=f"po{qc}{on}_{t}")
                    for hc in range(2):
                        nc.tensor.matmul(
                            pso[:],
                            ctx_chunks[t][hc][:, qc * 128:(qc + 1) * 128],
                            wo_t[:, hc * D + on * TW:
                                 hc * D + (on + 1) * TW],
                            start=(hc == 0), stop=(hc == 1),
                        )
                    # for the last tile split the PSUM->fp16 casts across
                    # DVE and ACT (both idle by then) to shorten the drain
                    if t == NT - 1 and on == 1:
                        nc.scalar.copy(ost[:, on * TW:(on + 1) * TW],
                                       pso[:])
                    else:
                        nc.vector.tensor_copy(ost[:, on * TW:(on + 1) * TW],
                                              pso[:])
                    # outputs ride the lightly-loaded ACT ring (the sync
                    # ring carries next-phase x loads)
                    row = t * TW + qc * 128
                    nc.scalar.dma_start(
                        out_d[row:row + 128, on * TW:(on + 1) * TW],
                        ost[:, on * TW:(on + 1) * TW])

            def c_list(t):
                return [lambda qc=qc, t=t: o_task(qc, t) for qc in range(4)]

            bs = {}
            for t in range(NT):
                for hp in range(2):
                    bs[(t, hp)] = b_stream(t, hp)
            # tile-3/hp0 split: [alloc, S0, (S1,P0)..(S11,P10)] goes a phase
            # early; [S12..S15, P11..P15, norm] stays.
            cut = 2 + 2 * 11
            phases = [
                ([dma_task(0)] + a_mm(0) + [dma_task(1)], [], []),
                (a_mm(1) + [dma_task(2)], bs[(0, 0)] + bs[(0, 1)], []),
                (a_mm(2) + [dma_task(3)], bs[(1, 0)] + bs[(1, 1)], c_list(0)),
                (a_mm(3), bs[(2, 0)] + bs[(2, 1)] + bs[(3, 0)][:cut],
                 c_list(1)),
                ([], bs[(3, 0)][cut:] + bs[(3, 1)], c_list(2)),
                ([], [], c_list(3)),
            ]
            for la, lb, lc in phases:
                for task in _tail_merge(lb, _weighted_merge(lc, la)):
                    task()

    _split_sync_waits(nc)
    return nc


_NC = None
TRACE = False
LAST_RESULTS = None


def kernel(query, key, value, attn_mask, Wq, bq, Wk, bk, Wv, bv, Wo, bo):
    global _NC, LAST_RESULTS
    query = np.asarray(query, np.float32)
    key = np.asarray(key, np.float32)
    value = np.asarray(value, np.float32)
    attn_mask = np.asarray(attn_mask, np.float32)
    Wq, Wk, Wv, Wo = (np.asarray(w, np.float32) for w in (Wq, Wk, Wv, Wo))
    bq, bk, bv, bo = (np.asarray(b, np.float32) for b in (bq, bk, bv, bo))

    if _NC is None:
        _NC = _build()

    hdt = np.float16
    # S^T tile element (i, j): keep k-row i iff attn_mask[q=j, k=i] == 0
    tri = np.ascontiguousarray((attn_mask[:128, :128].T == 0).astype(hdt))

    def chunk_x(x):
        # [S, D] -> xT [D, S] -> [NT, 128, KPC*TW]: out[t, p, kc*TW+c] =
        # x[t*TW+c, kc*128+p]
        xt = x.T.astype(hdt).reshape(KPC, 128, NT, TW)
        return np.ascontiguousarray(xt.transpose(2, 1, 0, 3)).reshape(
            NT, 128, KPC * TW)

    def chunk_w(w):
        # [D, DOUT] -> [128, KPC*DOUT]: out[p, kc*DOUT+m] = w[kc*128+p, m]
        return np.ascontiguousarray(
            w.astype(hdt).reshape(KPC, 128, DOUT).transpose(1, 0, 2)).reshape(
            128, KPC * DOUT)

    def chunk_wo(w):
        # [DOUT, D] -> [128, 2*D]
        return np.ascontiguousarray(
            w.astype(hdt).reshape(2, 128, D).transpose(1, 0, 2)).reshape(
            128, 2 * D)

    xT = {}
    for b in range(B):
        xT[("q", b)] = chunk_x(query[b])
        xT[("k", b)] = chunk_x(key[b])
        xT[("v", b)] = chunk_x(value[b])

    in_maps = []
    for c in range(NCORE):
        b, g = divmod(c, NCORE // B)
        sl = slice(g * DOUT, (g + 1) * DOUT)
        in_maps.append({
            "xqT": xT[("q", b)], "xkT": xT[("k", b)], "xvT": xT[("v", b)],
            "wq": chunk_w(Wq[:, sl]),
            "wk": chunk_w(Wk[:, sl]),
            "wv": chunk_w(Wv[:, sl]),
            "wo": chunk_wo(Wo[sl, :]),
            "bq": np.ascontiguousarray(bq[sl])[:, None],
            "bk": np.ascontiguousarray(bk[sl])[:, None],
            "tri": tri,
        })

    res = run_bass_kernel_spmd(_NC, in_maps, core_ids=list(range(NCORE)),
                               trace=TRACE)
    LAST_RESULTS = res

    extra = (bv @ Wo + bo).astype(np.float32)
    out = np.empty((B, S, D), np.float32)
    for b in range(B):
        acc = res.results[b * 4]["out"].astype(np.float32).copy()
        for g in range(1, NCORE // B):
            acc += res.results[b * 4 + g]["out"]
        out[b] = acc + extra
    return out
